# revision 36
# baseline (speedup 1.0000x reference)
"""Trainium2 Bass kernel for a DoReFa-quantized ResNet BasicBlock (inference).

Reference computation (all fp32):
    out = qact(bn2(conv3x3(qact(bn1(conv3x3(x, qw(w1)))), qw(w2))) + x)
with qw = 4-bit DoReFa weight quant, qact = 4-bit activation quant,
x: (64, 128, 56, 56), convs 128->128 stride 1 pad 1.

Sharding: data-parallel over the batch dim, 8 images per NeuronCore on 8 cores.

Per-core kernel design:
  * NCHW with C=128 on SBUF partitions, flattened zero-padded image rows in
    the free dim; a 3x3 conv = shifted 128x128 matmuls accumulated in PSUM
    (8-row chunks, one PSUM bank each).
  * Dual row pitches: conv1/x use 58-wide rows (464-col matmuls, minimal
    padding); act1/conv2 use 64-wide rows so the fp8 DoubleRow pair stride
    (2*64 = 128 B) satisfies the %16 rule. The activation-quantize round op
    bridges the two layouts for free via strided APs.
  * Quantized weights are exact small integers (15*w_q odd in [-15,15]) and
    activations are 15*a in {0..15} (exact in fp8e4m3) -> conv2 is bit-exact
    integer arithmetic in 5 matmuls per chunk: 3 fp8 DoubleRow matmuls for
    the (dy=-1,+1) tap pairs, a 4th DoubleRow pairing (dy=0,dx=-1)+(dx=+1)
    against a 16-byte-aligned shifted duplicate of act1 (produced by a second
    VectorE round-op), and 1 normal fp8 matmul for the center tap.
  * Conv1 runs in the PE's float32r mode (fp32 exponent, 12-bit significand,
    round-to-nearest; probed on HW) at 1 col/cycle - 4x faster than fp32.
  * BN folds to a per-channel affine applied by ScalarE out of PSUM; DoReFa
    staircase = tensor_scalar clip (max,min) + round-half-even via the +2^23
    fp32 trick on VectorE (bit-matches jnp.round).
  * Software-pipelined emission (conv1 of image n+1 ahead of conv2 of image
    n); all HBM transfers contiguous (staging tiles + VectorE pad insert);
    ~30 warm-up matmuls during the head DMA window pre-trip the PE HAM clock
    gate. A post-Tile pass splits multi-semaphore waits onto same-engine
    NoOps (this walrus encodes at most one sync wait per instruction).

Measured (8 cores, NTFF profile): ~204 us HW exec, rel L2 err ~8e-3
(~0.7% of outputs off by one 1/15 quantization step; PE >99% packed in its
window; 98 matmuls per image per core).
"""

import os
import sys

import numpy as np

for _p in ("/opt/trn_rl_repo", "/opt/pypackages"):
    if _p not in sys.path and os.path.isdir(_p):
        sys.path.insert(0, _p)

import ml_dtypes  # noqa: E402

# ---------------------------------------------------------------- constants
B, C, H, W = 64, 128, 56, 56
N_CORES = 8
BPC = B // N_CORES          # images per core
WP = W + 2                  # conv1/x padded row length (58)
WP2 = 64                    # conv2/act1 padded row length (58 used + 6 dead; 2*WP2 % 16 == 0 for DoubleRow)
HPAD = H + 2                # padded rows        (58)
IMG = WP * HPAD             # x-layout padded image elems (3364)
IMG2 = WP2 * HPAD           # act1-layout padded image elems (3712)
BUF = IMG + 4               # x/v1 buffer
BUF2 = IMG2 + 4             # act1/v2 buffer
ACT_D = 3726                # shifted act1 copy offset; pair step D+2 %16==0
ABUF = ACT_D + BUF2         # act1 tile width (original + shifted copy)
XB = 1                      # x / v / out buffers: image base offset
AB = 2                      # act1 buffer: base offset (keeps bf16 dest 4B aligned)
RPC = 8                     # padded rows per PSUM chunk
NCHUNK = H // RPC           # 7 chunks cover output rows 1..56
FREE = RPC * WP             # 464 free elems per conv1 matmul
FREE2 = RPC * WP2           # 512 free elems per conv2 matmul (one PSUM bank)
MAGIC = float(2**23)        # fp32 round-to-nearest-even magic constant
EPS = 1e-5

# conv1 input mode: "f32r" = single fp32r matmul per tap (fast; reduced-
# precision PE mode), "hilo" = fp16 hi+lo split (2 matmuls per tap, ~2^-22).
CONV1_MODE = os.environ.get("K_CONV1_MODE", "f32r")

_CACHE = {}


# ---------------------------------------------------------------- host math
def _quant_weight_int(w):
    """Return 15*quantize_weight(w, 4) which is an exact odd integer in
    [-15, 15], as float32. Mirrors reference elementwise fp32 ops; tanh is
    computed in f64 and rounded (closest to any correctly-rounded f32 tanh)."""
    wt = np.tanh(w.astype(np.float64)).astype(np.float32)
    m = np.float32(np.abs(wt).max())
    wtn = wt / (np.float32(2.0) * m) + np.float32(0.5)      # [0, 1]
    q = np.round(wtn * np.float32(15.0)).astype(np.float32)  # {0..15}, half-even
    return np.float32(2.0) * q - np.float32(15.0)            # odd ints [-15,15]


def _bn_affine(gamma, beta, mean, var):
    """Per-channel (scale, bias) with bn(y) = scale*y + bias, in f64."""
    inv = 1.0 / np.sqrt(var.astype(np.float64) + EPS)
    s = gamma.astype(np.float64) * inv
    b = beta.astype(np.float64) - mean.astype(np.float64) * s
    return s, b


def _lhsT_taps(w_int):
    """[oc, ic, 3, 3] -> [ic, 9*oc] stationary-operand layout (tap-major)."""
    # lhsT for tap t lives at columns [t*128, (t+1)*128), laid out [ic, oc]
    t = np.transpose(w_int, (2, 3, 1, 0)).reshape(9, C, C)   # [tap, ic, oc]
    return np.transpose(t, (1, 0, 2)).reshape(C, 9 * C)


# ---------------------------------------------------------------- bass build
def _split_multiwaits(nc, mybir):
    """Walrus in this toolchain encodes at most ONE sync wait per instruction.

    Tile emits instructions with several on_wait entries; hoist all but one
    onto same-engine NoOps placed immediately before the instruction (the
    sequencer executes them in order, so semantics are unchanged)."""
    nid = 0
    for fn in nc.m.functions:
        for blk in fn.blocks:
            out = []
            changed = False
            for ins in blk.instructions:
                si = ins.sync_info
                if si is not None and len(si.on_wait) > 1:
                    waits = list(si.on_wait)
                    for w in waits[:-1]:
                        nid += 1
                        nop = mybir.InstNoOp(name=f"I-wfix-{nid}",
                                             engine=ins.engine)
                        nop.sync_info = mybir.SyncInfo(on_wait=[w],
                                                       on_update=[])
                        out.append(nop)
                    ins.sync_info = mybir.SyncInfo(
                        on_wait=[waits[-1]], on_update=list(si.on_update))
                    changed = True
                out.append(ins)
            if changed:
                blk.instructions = out


def _build_module(apply_wfix=True):
    import concourse.bass as bass
    import concourse.mybir as mybir
    import concourse.tile as tile
    from contextlib import ExitStack

    f32 = mybir.dt.float32
    f16 = mybir.dt.float16
    bf16 = mybir.dt.bfloat16
    AF = mybir.ActivationFunctionType
    OP = mybir.AluOpType

    nc = bass.Bass("TRN2", target_bir_lowering=False, debug=False,
                   num_devices=N_CORES)

    f32r = mybir.dt.float32r
    u32 = mybir.dt.uint32

    x_dtype = f32r if CONV1_MODE == "f32r" else f32
    x_d = nc.dram_tensor("x15", [BPC, C, H, W], x_dtype, kind="ExternalInput")
    # conv2 weights fp8: 3 DoubleRow pair blocks [2,128] (dy=-1/+1 per dx),
    # then 3 dy=0 single taps
    f8 = mybir.dt.float8e4
    w2p_d = nc.dram_tensor("w2p", [C, 9 * C], f8, kind="ExternalInput")
    # conv1 weights as f32 (for the f32r matmul path)
    w1r_d = nc.dram_tensor("w1r", [C, 9 * C], f32r, kind="ExternalInput")
    # columns: [sc1, bi1, sc2, bi2]
    bn_d = nc.dram_tensor("bnv", [C, 4], f32, kind="ExternalInput")
    out_d = nc.dram_tensor("out", [BPC, C, H, W], f32, kind="ExternalOutput")

    lo = XB + WP            # first valid (row 1) element in XB-based buffers
    hi = XB + (HPAD - 1) * WP  # one past row 56 (= start of pad row 57)

    with tile.TileContext(nc) as tc, ExitStack() as ctx:
        const = ctx.enter_context(tc.tile_pool(name="const", bufs=1))
        sb = ctx.enter_context(tc.tile_pool(name="sb", bufs=2))
        xp = ctx.enter_context(tc.tile_pool(name="xp", bufs=3))
        ps = ctx.enter_context(tc.tile_pool(name="ps", bufs=4, space="PSUM"))

        # order: conv1 weights first (first-matmul critical path), then BN
        # vectors, then conv2 weights (not needed until ~40us in)
        w1r_sb = const.tile([C, 9 * C], f32r)
        if CONV1_MODE == "f32r":
            for q in range(3):
                nc.sync.dma_start(w1r_sb[:, q * 3 * C:(q + 1) * 3 * C],
                                  w1r_d.ap()[:, q * 3 * C:(q + 1) * 3 * C])
        bn_sb = const.tile([C, 4], f32)
        nc.sync.dma_start(bn_sb[:], bn_d.ap())
        w2p_sb = const.tile([C, 9 * C], f8)
        nc.sync.dma_start(w2p_sb[:], w2p_d.ap())
        if CONV1_MODE == "f32r":
            # HAM warm-up: ~30 throwaway matmuls on the just-landed weights
            # fill the otherwise-idle head so real matmuls start at 2.4 GHz
            pwarm = ps.tile([C, FREE], f32, tag="p1", name="pwarm")
            for _ in range(30):
                nc.tensor.matmul(pwarm[:, 0:C], lhsT=w1r_sb[:, 0:C],
                                 rhs=w1r_sb[:, 0:C], start=True, stop=True)
        sc1_sb = bn_sb[:, 0:1]
        bi1_sb = bn_sb[:, 1:2]
        sc2_sb = bn_sb[:, 2:3]
        bi2_sb = bn_sb[:, 3:4]

        def emit_load_conv1(n):
            """Load image n, run conv1 + bn1 + qact; returns (x, act1)."""
            x = xp.tile([C, BUF], f32, tag="x", name=f"x_{n}")
            xw = x.bitcast(f32r) if CONV1_MODE == "f32r" else x
            xwr = xw[:, XB:XB + IMG].rearrange("p (h w) -> p h w", w=WP)
            # zero pad borders + slack (everything outside the DMA interior)
            xr0 = x[:, XB:XB + IMG].rearrange("p (h w) -> p h w", w=WP)
            nc.gpsimd.memset(x[:, 0:XB + WP + 1], 0.0)          # slack+row0
            nc.gpsimd.memset(x[:, XB + (HPAD - 1) * WP:BUF], 0.0)  # row57+slack
            nc.gpsimd.memset(xr0[:, 1:57, 0], 0.0)               # left pad col
            nc.gpsimd.memset(xr0[:, 1:57, 57], 0.0)              # right pad col
            # contiguous quarter DMAs into staging (128 descriptors each,
            # vs 1792 for strided pad-layout writes), then DVE pad-insert
            xs = sb.tile([C, H * W], x_dtype, tag="xs", name=f"xs_{n}")
            xsr = xs.rearrange("p (h w) -> p h w", w=W)
            xd_flat = x_d.ap()[n].rearrange("p h w -> p (h w)")
            for q in range(4):
                r0, r1 = 1 + 14 * q, 15 + 14 * q
                nc.sync.dma_start(xs[:, (r0 - 1) * W:(r1 - 1) * W],
                                  xd_flat[:, (r0 - 1) * W:(r1 - 1) * W])
                nc.vector.tensor_copy(
                    xwr[:, r0:r1, 1:57],
                    xsr[:, r0 - 1:r1 - 1, :])

            v1 = sb.tile([C, BUF], f32, tag="v1", name=f"v1_{n}")
            for cch in range(NCHUNK):
                p1 = ps.tile([C, FREE], f32, tag="p1", name=f"p1_{n}_{cch}")
                r0 = 1 + RPC * cch
                for t9 in range(9):
                    dy, dx = t9 // 3 - 1, t9 % 3 - 1
                    off = XB + (r0 + dy) * WP + dx
                    nc.tensor.matmul(
                        p1[:],
                        lhsT=w1r_sb[:, t9 * C:(t9 + 1) * C],
                        rhs=xw[:, off:off + FREE],
                        start=(t9 == 0), stop=(t9 == 8))
                dst = v1[:, XB + r0 * WP:XB + r0 * WP + FREE]
                nc.scalar.activation(dst, p1[:], AF.Identity,
                                     bias=bi1_sb, scale=sc1_sb)

            # qact: clip to [0,15] then round (kept as 15*act, fp8 exact).
            # The round op also re-layouts W58 (v1) -> W64 (act1) rows.
            nc.vector.tensor_scalar(v1[:, lo:hi], v1[:, lo:hi],
                                    0.0, 15.0, op0=OP.max, op1=OP.min)
            act1 = sb.tile([C, ABUF], f8, tag="act1", name=f"act1_{n}")
            v1r = v1[:, XB:XB + IMG].rearrange("p (h w) -> p h w", w=WP)
            ar = act1[:, AB:AB + IMG2].rearrange("p (h w) -> p h w", w=WP2)
            nc.vector.tensor_scalar(ar[:, 1:57, 0:WP], v1r[:, 1:57, 0:WP],
                                    MAGIC, MAGIC, op0=OP.add, op1=OP.subtract)
            nc.gpsimd.memset(act1[:, 0:AB + WP2 + 1], 0.0)
            nc.gpsimd.memset(act1[:, AB + (HPAD - 1) * WP2:BUF2], 0.0)
            nc.gpsimd.memset(ar[:, 1:57, 0], 0.0)
            nc.gpsimd.memset(ar[:, 1:57, 57:64], 0.0)
            # shifted duplicate of act1 (for the dy=0 DoubleRow pair):
            # second round-op writing at +ACT_D, plus its own zero borders
            ar2 = act1[:, ACT_D + AB:ACT_D + AB + IMG2].rearrange(
                "p (h w) -> p h w", w=WP2)
            nc.vector.tensor_scalar(ar2[:, 1:57, 0:WP], v1r[:, 1:57, 0:WP],
                                    MAGIC, MAGIC, op0=OP.add, op1=OP.subtract)
            nc.gpsimd.memset(act1[:, ACT_D:ACT_D + AB + WP2 + 1], 0.0)
            nc.gpsimd.memset(act1[:, ACT_D + AB + (HPAD - 1) * WP2:ABUF], 0.0)
            nc.gpsimd.memset(ar2[:, 1:57, 0], 0.0)
            nc.gpsimd.memset(ar2[:, 1:57, 57:64], 0.0)
            return x, act1

        def emit_conv2_out(n, x, act1):
            """conv2 + bn2 + residual + qact for image n, DMA result out."""
            v2 = sb.tile([C, BUF2], f32, tag="v2", name=f"v2_{n}")
            for cch in range(NCHUNK):
                p2 = ps.tile([C, FREE2], f32, tag="p2", name=f"p2_{n}_{cch}")
                r0 = 1 + RPC * cch
                for dxi, dx in enumerate((-1, 0, 1)):
                    # DoubleRow: taps (dy=-1,dx) + (dy=+1,dx) in one matmul;
                    # pair stride = 2*WP2 = 128 fp8 bytes (%16 == 0)
                    off_a = AB + (r0 - 1) * WP2 + dx
                    mv = bass.AP(tensor=act1.tensor, offset=off_a,
                                 ap=[[ABUF, C], [2 * WP2, 2], [1, FREE2]])
                    wpair = w2p_sb[:, dxi * 2 * C:(dxi + 1) * 2 * C].rearrange(
                        "p (two m) -> p two m", two=2)
                    nc.tensor.matmul(p2[:], lhsT=wpair, rhs=mv,
                                     perf_mode=mybir.MatmulPerfMode.DoubleRow,
                                     start=(dxi == 0), stop=False)
                # 4th DoubleRow: (dy=0,dx=-1) from the original + (dy=0,dx=+1)
                # from the shifted copy -> pair step ACT_D + 2 (16-aligned)
                off_a = AB + r0 * WP2 - 1
                mv = bass.AP(tensor=act1.tensor, offset=off_a,
                             ap=[[ABUF, C], [ACT_D + 2, 2], [1, FREE2]])
                wpair = w2p_sb[:, 6 * C:8 * C].rearrange(
                    "p (two m) -> p two m", two=2)
                nc.tensor.matmul(p2[:], lhsT=wpair, rhs=mv,
                                 perf_mode=mybir.MatmulPerfMode.DoubleRow,
                                 start=False, stop=False)
                # remaining single: (dy=0, dx=0)
                off = AB + r0 * WP2
                nc.tensor.matmul(p2[:], lhsT=w2p_sb[:, 8 * C:9 * C],
                                 rhs=act1[:, off:off + FREE2],
                                 start=False, stop=True)
                dst = v2[:, XB + r0 * WP2:XB + r0 * WP2 + FREE2]
                nc.scalar.activation(dst, p2[:], AF.Identity,
                                     bias=bi2_sb, scale=sc2_sb)

            # + residual (x buffer holds 15*x), then qact, then /15 —
            # in two half-image pieces so the first out-DMA overlaps the
            # second half's elementwise tail.
            inv15 = float(np.float32(1.0) / np.float32(15.0))
            vr = v2[:, XB:XB + IMG2].rearrange("p (h w) -> p h w", w=WP2)
            xr = x[:, XB:XB + IMG].rearrange("p (h w) -> p h w", w=WP)
            ost = sb.tile([C, H * W], f32, tag="ost", name=f"ost_{n}")
            ostr = ost.rearrange("p (h w) -> p h w", w=W)
            od_flat = out_d.ap()[n].rearrange("p h w -> p (h w)")
            for q in range(4):
                r0, r1 = 1 + 14 * q, 15 + 14 * q
                vq = vr[:, r0:r1, 0:WP]      # [14, 58] rows of the W64 buffer
                nc.vector.tensor_add(vq, vq, xr[:, r0:r1, 0:WP])
                nc.vector.tensor_scalar(vq, vq,
                                        0.0, 15.0, op0=OP.max, op1=OP.min)
                nc.vector.tensor_scalar(vq, vq, MAGIC, MAGIC,
                                        op0=OP.add, op1=OP.subtract)
                # final *1/15 writes straight into the contiguous staging
                nc.vector.tensor_scalar_mul(ostr[:, r0 - 1:r1 - 1, :],
                                            vr[:, r0:r1, 1:57], inv15)
                nc.sync.dma_start(od_flat[:, (r0 - 1) * W:(r1 - 1) * W],
                                  ost[:, (r0 - 1) * W:(r1 - 1) * W])

        prev = None
        for s in range(BPC + 1):
            cur = emit_load_conv1(s) if s < BPC else None
            if prev is not None:
                emit_conv2_out(s - 1, *prev)
            prev = cur

    if apply_wfix:
        _split_multiwaits(nc, mybir)
    return nc


def _get_module(apply_wfix=True):
    key = ("nc", apply_wfix)
    if key not in _CACHE:
        _CACHE[key] = _build_module(apply_wfix)
    return _CACHE[key]


# ---------------------------------------------------------------- host entry
def _make_in_maps(x, w1, w2, gamma1, beta1, mean1, var1,
                  gamma2, beta2, mean2, var2):
    x15 = (np.float32(15.0) * np.asarray(x, np.float32))
    x15 = x15.reshape(N_CORES, BPC, C, H, W)

    w1i = _quant_weight_int(np.asarray(w1, np.float32))
    w2i = _quant_weight_int(np.asarray(w2, np.float32))
    w2t = _lhsT_taps(w2i)  # [C, 9*C], tap-major (t9 = (dy+1)*3 + dx+1)
    tap = lambda t9: w2t[:, t9 * C:(t9 + 1) * C]
    blocks = []
    for dxi in range(3):           # DR pairs: (dy=-1,dx) then (dy=+1,dx)
        blocks += [tap(dxi), tap(6 + dxi)]
    blocks += [tap(3), tap(5)]     # DR pair: (dy=0,dx=-1) + (dy=0,dx=+1)
    blocks.append(tap(4))          # single: (dy=0,dx=0)
    w2p = np.concatenate(blocks, axis=1).astype(ml_dtypes.float8_e4m3)

    s1, b1 = _bn_affine(np.asarray(gamma1, np.float32), np.asarray(beta1, np.float32),
                        np.asarray(mean1, np.float32), np.asarray(var1, np.float32))
    s2, b2 = _bn_affine(np.asarray(gamma2, np.float32), np.asarray(beta2, np.float32),
                        np.asarray(mean2, np.float32), np.asarray(var2, np.float32))
    # conv PSUM holds 225*conv (15x-or-15a input, 15w weights) -> want 15*bn:
    bnv = np.stack([s1 / 15.0, 15.0 * b1, s2 / 15.0, 15.0 * b2],
                   axis=1).astype(np.float32)  # [C, 4]

    w1r = _lhsT_taps(w1i).astype(np.float32)
    shared = {"w2p": w2p, "w1r": w1r, "bnv": bnv}
    return [{"x15": np.ascontiguousarray(x15[i]), **shared}
            for i in range(N_CORES)]


def kernel(**inputs):
    from concourse.bass_utils import run_bass_kernel_spmd

    nc = _get_module()
    in_maps = _make_in_maps(**inputs)
    res = run_bass_kernel_spmd(nc, in_maps, core_ids=list(range(N_CORES)))
    _CACHE["last_res"] = res
    out = np.concatenate([np.asarray(r["out"], np.float32)
                          for r in res.results], axis=0)
    return out.reshape(B, C, H, W)


# revision 37
# speedup vs baseline: 1.0037x; 1.0037x over previous
"""Trainium2 Bass kernel for a DoReFa-quantized ResNet BasicBlock (inference).

Reference computation (all fp32):
    out = qact(bn2(conv3x3(qact(bn1(conv3x3(x, qw(w1)))), qw(w2))) + x)
with qw = 4-bit DoReFa weight quant, qact = 4-bit activation quant,
x: (64, 128, 56, 56), convs 128->128 stride 1 pad 1.

Sharding: data-parallel over the batch dim, 8 images per NeuronCore on 8 cores.

Per-core kernel design:
  * NCHW with C=128 on SBUF partitions, flattened zero-padded image rows in
    the free dim; a 3x3 conv = shifted 128x128 matmuls accumulated in PSUM
    (8-row chunks, one PSUM bank each).
  * Dual row pitches: conv1/x use 58-wide rows (464-col matmuls, minimal
    padding); act1/conv2 use 64-wide rows so the fp8 DoubleRow pair stride
    (2*64 = 128 B) satisfies the %16 rule. The activation-quantize round op
    bridges the two layouts for free via strided APs.
  * Quantized weights are exact small integers (15*w_q odd in [-15,15]) and
    activations are 15*a in {0..15} (exact in fp8e4m3) -> conv2 is bit-exact
    integer arithmetic in 5 matmuls per chunk: 3 fp8 DoubleRow matmuls for
    the (dy=-1,+1) tap pairs, a 4th DoubleRow pairing (dy=0,dx=-1)+(dx=+1)
    against a 16-byte-aligned shifted duplicate of act1 (produced by a second
    VectorE round-op), and 1 normal fp8 matmul for the center tap.
  * Conv1 runs in the PE's float32r mode (fp32 exponent, 12-bit significand,
    round-to-nearest; probed on HW) at 1 col/cycle - 4x faster than fp32.
  * BN folds to a per-channel affine applied by ScalarE out of PSUM; DoReFa
    staircase = tensor_scalar clip (max,min) + round-half-even via the +2^23
    fp32 trick on VectorE (bit-matches jnp.round).
  * Software-pipelined emission (conv1 of image n+1 ahead of conv2 of image
    n); all HBM transfers contiguous (staging tiles + VectorE pad insert);
    ~30 warm-up matmuls during the head DMA window pre-trip the PE HAM clock
    gate. A post-Tile pass splits multi-semaphore waits onto same-engine
    NoOps (this walrus encodes at most one sync wait per instruction).

Measured (8 cores, NTFF profile): ~204 us HW exec, rel L2 err ~8e-3
(~0.7% of outputs off by one 1/15 quantization step; PE >99% packed in its
window; 98 matmuls per image per core).
"""

import os
import sys

import numpy as np

for _p in ("/opt/trn_rl_repo", "/opt/pypackages"):
    if _p not in sys.path and os.path.isdir(_p):
        sys.path.insert(0, _p)

import ml_dtypes  # noqa: E402

# ---------------------------------------------------------------- constants
B, C, H, W = 64, 128, 56, 56
N_CORES = 8
BPC = B // N_CORES          # images per core
WP = W + 2                  # conv1/x padded row length (58)
WP2 = 64                    # conv2/act1 padded row length (58 used + 6 dead; 2*WP2 % 16 == 0 for DoubleRow)
HPAD = H + 2                # padded rows        (58)
IMG = WP * HPAD             # x-layout padded image elems (3364)
IMG2 = WP2 * HPAD           # act1-layout padded image elems (3712)
BUF = IMG + 4               # x/v1 buffer
BUF2 = IMG2 + 4             # act1/v2 buffer
ACT_D = 3726                # shifted act1 copy offset; pair step D+2 %16==0
ABUF = ACT_D + BUF2         # act1 tile width (original + shifted copy)
XB = 1                      # x / v / out buffers: image base offset
AB = 2                      # act1 buffer: base offset (keeps bf16 dest 4B aligned)
RPC = 8                     # padded rows per PSUM chunk
NCHUNK = H // RPC           # 7 chunks cover output rows 1..56
FREE = RPC * WP             # 464 free elems per conv1 matmul
FREE2 = RPC * WP2           # 512 free elems per conv2 matmul (one PSUM bank)
MAGIC = float(2**23)        # fp32 round-to-nearest-even magic constant
EPS = 1e-5

# conv1 input mode: "f32r" = single fp32r matmul per tap (fast; reduced-
# precision PE mode), "hilo" = fp16 hi+lo split (2 matmuls per tap, ~2^-22).
CONV1_MODE = os.environ.get("K_CONV1_MODE", "f32r")

_CACHE = {}


# ---------------------------------------------------------------- host math
def _quant_weight_int(w):
    """Return 15*quantize_weight(w, 4) which is an exact odd integer in
    [-15, 15], as float32. Mirrors reference elementwise fp32 ops; tanh is
    computed in f64 and rounded (closest to any correctly-rounded f32 tanh)."""
    wt = np.tanh(w.astype(np.float64)).astype(np.float32)
    m = np.float32(np.abs(wt).max())
    wtn = wt / (np.float32(2.0) * m) + np.float32(0.5)      # [0, 1]
    q = np.round(wtn * np.float32(15.0)).astype(np.float32)  # {0..15}, half-even
    return np.float32(2.0) * q - np.float32(15.0)            # odd ints [-15,15]


def _bn_affine(gamma, beta, mean, var):
    """Per-channel (scale, bias) with bn(y) = scale*y + bias, in f64."""
    inv = 1.0 / np.sqrt(var.astype(np.float64) + EPS)
    s = gamma.astype(np.float64) * inv
    b = beta.astype(np.float64) - mean.astype(np.float64) * s
    return s, b


def _lhsT_taps(w_int):
    """[oc, ic, 3, 3] -> [ic, 9*oc] stationary-operand layout (tap-major)."""
    # lhsT for tap t lives at columns [t*128, (t+1)*128), laid out [ic, oc]
    t = np.transpose(w_int, (2, 3, 1, 0)).reshape(9, C, C)   # [tap, ic, oc]
    return np.transpose(t, (1, 0, 2)).reshape(C, 9 * C)


# ---------------------------------------------------------------- bass build
def _split_multiwaits(nc, mybir):
    """Walrus in this toolchain encodes at most ONE sync wait per instruction.

    Tile emits instructions with several on_wait entries; hoist all but one
    onto same-engine NoOps placed immediately before the instruction (the
    sequencer executes them in order, so semantics are unchanged)."""
    nid = 0
    for fn in nc.m.functions:
        for blk in fn.blocks:
            out = []
            changed = False
            for ins in blk.instructions:
                si = ins.sync_info
                if si is not None and len(si.on_wait) > 1:
                    waits = list(si.on_wait)
                    for w in waits[:-1]:
                        nid += 1
                        nop = mybir.InstNoOp(name=f"I-wfix-{nid}",
                                             engine=ins.engine)
                        nop.sync_info = mybir.SyncInfo(on_wait=[w],
                                                       on_update=[])
                        out.append(nop)
                    ins.sync_info = mybir.SyncInfo(
                        on_wait=[waits[-1]], on_update=list(si.on_update))
                    changed = True
                out.append(ins)
            if changed:
                blk.instructions = out


def _build_module(apply_wfix=True):
    import concourse.bass as bass
    import concourse.mybir as mybir
    import concourse.tile as tile
    from contextlib import ExitStack

    f32 = mybir.dt.float32
    f16 = mybir.dt.float16
    bf16 = mybir.dt.bfloat16
    AF = mybir.ActivationFunctionType
    OP = mybir.AluOpType

    nc = bass.Bass("TRN2", target_bir_lowering=False, debug=False,
                   num_devices=N_CORES)

    f32r = mybir.dt.float32r
    u32 = mybir.dt.uint32

    x_dtype = f32r if CONV1_MODE == "f32r" else f32
    x_d = nc.dram_tensor("x15", [BPC, C, H, W], x_dtype, kind="ExternalInput")
    # conv2 weights fp8: 3 DoubleRow pair blocks [2,128] (dy=-1/+1 per dx),
    # then 3 dy=0 single taps
    f8 = mybir.dt.float8e4
    w2p_d = nc.dram_tensor("w2p", [C, 9 * C], f8, kind="ExternalInput")
    # conv1 weights as f32 (for the f32r matmul path)
    w1r_d = nc.dram_tensor("w1r", [C, 9 * C], f32r, kind="ExternalInput")
    # columns: [sc1, bi1, sc2, bi2]
    bn_d = nc.dram_tensor("bnv", [C, 4], f32, kind="ExternalInput")
    out_d = nc.dram_tensor("out", [BPC, C, H, W], f32, kind="ExternalOutput")

    lo = XB + WP            # first valid (row 1) element in XB-based buffers
    hi = XB + (HPAD - 1) * WP  # one past row 56 (= start of pad row 57)

    with tile.TileContext(nc) as tc, ExitStack() as ctx:
        const = ctx.enter_context(tc.tile_pool(name="const", bufs=1))
        sb = ctx.enter_context(tc.tile_pool(name="sb", bufs=2))
        xp = ctx.enter_context(tc.tile_pool(name="xp", bufs=3))
        ps = ctx.enter_context(tc.tile_pool(name="ps", bufs=4, space="PSUM"))

        # order: conv1 weights first (first-matmul critical path), then BN
        # vectors, then conv2 weights (not needed until ~40us in)
        w1r_sb = const.tile([C, 9 * C], f32r)
        if CONV1_MODE == "f32r":
            for q in range(3):
                nc.sync.dma_start(w1r_sb[:, q * 3 * C:(q + 1) * 3 * C],
                                  w1r_d.ap()[:, q * 3 * C:(q + 1) * 3 * C])
        bn_sb = const.tile([C, 4], f32)
        nc.sync.dma_start(bn_sb[:], bn_d.ap())
        w2p_sb = const.tile([C, 9 * C], f8)
        nc.sync.dma_start(w2p_sb[:], w2p_d.ap())
        if CONV1_MODE == "f32r":
            # HAM warm-up: ~30 throwaway matmuls on the just-landed weights
            # fill the otherwise-idle head so real matmuls start at 2.4 GHz
            pwarm = ps.tile([C, FREE], f32, tag="p1", name="pwarm")
            for _ in range(30):
                nc.tensor.matmul(pwarm[:, 0:C], lhsT=w1r_sb[:, 0:C],
                                 rhs=w1r_sb[:, 0:C], start=True, stop=True)
        sc1_sb = bn_sb[:, 0:1]
        bi1_sb = bn_sb[:, 1:2]
        sc2_sb = bn_sb[:, 2:3]
        bi2_sb = bn_sb[:, 3:4]

        def emit_load_conv1(n):
            """Load image n, run conv1 + bn1 + qact; returns (x, act1)."""
            x = xp.tile([C, BUF], f32, tag="x", name=f"x_{n}")
            xw = x.bitcast(f32r) if CONV1_MODE == "f32r" else x
            xwr = xw[:, XB:XB + IMG].rearrange("p (h w) -> p h w", w=WP)
            # zero pad borders + slack (everything outside the DMA interior)
            xr0 = x[:, XB:XB + IMG].rearrange("p (h w) -> p h w", w=WP)
            nc.gpsimd.memset(x[:, 0:XB + WP + 1], 0.0)          # slack+row0
            nc.gpsimd.memset(x[:, XB + (HPAD - 1) * WP:BUF], 0.0)  # row57+slack
            nc.gpsimd.memset(xr0[:, 1:57, 0], 0.0)               # left pad col
            nc.gpsimd.memset(xr0[:, 1:57, 57], 0.0)              # right pad col
            # contiguous quarter DMAs into staging (128 descriptors each,
            # vs 1792 for strided pad-layout writes), then DVE pad-insert
            xs = sb.tile([C, H * W], x_dtype, tag="xs", name=f"xs_{n}")
            xsr = xs.rearrange("p (h w) -> p h w", w=W)
            xd_flat = x_d.ap()[n].rearrange("p h w -> p (h w)")
            for q in range(4):
                r0, r1 = 1 + 14 * q, 15 + 14 * q
                nc.sync.dma_start(xs[:, (r0 - 1) * W:(r1 - 1) * W],
                                  xd_flat[:, (r0 - 1) * W:(r1 - 1) * W])
                nc.vector.tensor_copy(
                    xwr[:, r0:r1, 1:57],
                    xsr[:, r0 - 1:r1 - 1, :])

            v1 = sb.tile([C, BUF], f32, tag="v1", name=f"v1_{n}")
            for cch in range(NCHUNK):
                p1 = ps.tile([C, FREE], f32, tag="p1", name=f"p1_{n}_{cch}")
                r0 = 1 + RPC * cch
                for t9 in range(9):
                    dy, dx = t9 // 3 - 1, t9 % 3 - 1
                    off = XB + (r0 + dy) * WP + dx
                    nc.tensor.matmul(
                        p1[:],
                        lhsT=w1r_sb[:, t9 * C:(t9 + 1) * C],
                        rhs=xw[:, off:off + FREE],
                        start=(t9 == 0), stop=(t9 == 8))
                dst = v1[:, XB + r0 * WP:XB + r0 * WP + FREE]
                nc.scalar.activation(dst, p1[:], AF.Identity,
                                     bias=bi1_sb, scale=sc1_sb)

            # qact: clip to [0,15] then round (kept as 15*act, fp8 exact).
            # The round op also re-layouts W58 (v1) -> W64 (act1) rows.
            nc.vector.tensor_scalar(v1[:, lo:hi], v1[:, lo:hi],
                                    0.0, 15.0, op0=OP.max, op1=OP.min)
            act1 = sb.tile([C, ABUF], f8, tag="act1", name=f"act1_{n}")
            v1r = v1[:, XB:XB + IMG].rearrange("p (h w) -> p h w", w=WP)
            ar = act1[:, AB:AB + IMG2].rearrange("p (h w) -> p h w", w=WP2)
            nc.vector.tensor_scalar(ar[:, 1:57, 0:WP], v1r[:, 1:57, 0:WP],
                                    MAGIC, MAGIC, op0=OP.add, op1=OP.subtract)
            nc.gpsimd.memset(act1[:, 0:AB + WP2 + 1], 0.0)
            nc.gpsimd.memset(act1[:, AB + (HPAD - 1) * WP2:BUF2], 0.0)
            nc.gpsimd.memset(ar[:, 1:57, 0], 0.0)
            nc.gpsimd.memset(ar[:, 1:57, 57:64], 0.0)
            # shifted duplicate of act1 (for the dy=0 DoubleRow pair):
            # second round-op writing at +ACT_D, plus its own zero borders
            ar2 = act1[:, ACT_D + AB:ACT_D + AB + IMG2].rearrange(
                "p (h w) -> p h w", w=WP2)
            nc.vector.tensor_scalar(ar2[:, 1:57, 0:WP], v1r[:, 1:57, 0:WP],
                                    MAGIC, MAGIC, op0=OP.add, op1=OP.subtract)
            nc.gpsimd.memset(act1[:, ACT_D:ACT_D + AB + WP2 + 1], 0.0)
            nc.gpsimd.memset(act1[:, ACT_D + AB + (HPAD - 1) * WP2:ABUF], 0.0)
            nc.gpsimd.memset(ar2[:, 1:57, 0], 0.0)
            nc.gpsimd.memset(ar2[:, 1:57, 57:64], 0.0)
            return x, act1

        def emit_conv2_out(n, x, act1):
            """conv2 + bn2 + residual + qact for image n, DMA result out."""
            v2 = sb.tile([C, BUF2], f32, tag="v2", name=f"v2_{n}")
            inv15 = float(np.float32(1.0) / np.float32(15.0))
            vr = v2[:, XB:XB + IMG2].rearrange("p (h w) -> p h w", w=WP2)
            xr = x[:, XB:XB + IMG].rearrange("p (h w) -> p h w", w=WP)
            ost = sb.tile([C, H * W], f32, tag="ost", name=f"ost_{n}")
            ostr = ost.rearrange("p (h w) -> p h w", w=W)
            od_flat = out_d.ap()[n].rearrange("p h w -> p (h w)")

            def emit_quarter(q):
                # residual + qact + /15 + out-DMA for output rows 14q..14q+14;
                # emitted right after the last PSUM chunk covering them so the
                # scheduler drains the elementwise tail during the MM stream
                r0, r1 = 1 + 14 * q, 15 + 14 * q
                vq = vr[:, r0:r1, 0:WP]      # [14, 58] rows of the W64 buffer
                nc.vector.tensor_add(vq, vq, xr[:, r0:r1, 0:WP])
                nc.vector.tensor_scalar(vq, vq,
                                        0.0, 15.0, op0=OP.max, op1=OP.min)
                nc.vector.tensor_scalar(vq, vq, MAGIC, MAGIC,
                                        op0=OP.add, op1=OP.subtract)
                nc.vector.tensor_scalar_mul(ostr[:, r0 - 1:r1 - 1, :],
                                            vr[:, r0:r1, 1:57], inv15)
                nc.sync.dma_start(od_flat[:, (r0 - 1) * W:(r1 - 1) * W],
                                  ost[:, (r0 - 1) * W:(r1 - 1) * W])

            # quarter q is ready once chunk ceil((14*(q+1))/8)-1 is done
            quarter_after = {1: 0, 3: 1, 5: 2, 6: 3}
            for cch in range(NCHUNK):
                p2 = ps.tile([C, FREE2], f32, tag="p2", name=f"p2_{n}_{cch}")
                r0 = 1 + RPC * cch
                for dxi, dx in enumerate((-1, 0, 1)):
                    # DoubleRow: taps (dy=-1,dx) + (dy=+1,dx) in one matmul;
                    # pair stride = 2*WP2 = 128 fp8 bytes (%16 == 0)
                    off_a = AB + (r0 - 1) * WP2 + dx
                    mv = bass.AP(tensor=act1.tensor, offset=off_a,
                                 ap=[[ABUF, C], [2 * WP2, 2], [1, FREE2]])
                    wpair = w2p_sb[:, dxi * 2 * C:(dxi + 1) * 2 * C].rearrange(
                        "p (two m) -> p two m", two=2)
                    nc.tensor.matmul(p2[:], lhsT=wpair, rhs=mv,
                                     perf_mode=mybir.MatmulPerfMode.DoubleRow,
                                     start=(dxi == 0), stop=False)
                # 4th DoubleRow: (dy=0,dx=-1) from the original + (dy=0,dx=+1)
                # from the shifted copy -> pair step ACT_D + 2 (16-aligned)
                off_a = AB + r0 * WP2 - 1
                mv = bass.AP(tensor=act1.tensor, offset=off_a,
                             ap=[[ABUF, C], [ACT_D + 2, 2], [1, FREE2]])
                wpair = w2p_sb[:, 6 * C:8 * C].rearrange(
                    "p (two m) -> p two m", two=2)
                nc.tensor.matmul(p2[:], lhsT=wpair, rhs=mv,
                                 perf_mode=mybir.MatmulPerfMode.DoubleRow,
                                 start=False, stop=False)
                # remaining single: (dy=0, dx=0)
                off = AB + r0 * WP2
                nc.tensor.matmul(p2[:], lhsT=w2p_sb[:, 8 * C:9 * C],
                                 rhs=act1[:, off:off + FREE2],
                                 start=False, stop=True)
                dst = v2[:, XB + r0 * WP2:XB + r0 * WP2 + FREE2]
                nc.scalar.activation(dst, p2[:], AF.Identity,
                                     bias=bi2_sb, scale=sc2_sb)
                if cch in quarter_after:
                    emit_quarter(quarter_after[cch])

            # + residual (x buffer holds 15*x), then qact, then /15 —
            # in two half-image pieces so the first out-DMA overlaps the
            # second half's elementwise tail.


        prev = None
        for s in range(BPC + 1):
            cur = emit_load_conv1(s) if s < BPC else None
            if prev is not None:
                emit_conv2_out(s - 1, *prev)
            prev = cur

    if apply_wfix:
        _split_multiwaits(nc, mybir)
    return nc


def _get_module(apply_wfix=True):
    key = ("nc", apply_wfix)
    if key not in _CACHE:
        _CACHE[key] = _build_module(apply_wfix)
    return _CACHE[key]


# ---------------------------------------------------------------- host entry
def _make_in_maps(x, w1, w2, gamma1, beta1, mean1, var1,
                  gamma2, beta2, mean2, var2):
    x15 = (np.float32(15.0) * np.asarray(x, np.float32))
    x15 = x15.reshape(N_CORES, BPC, C, H, W)

    w1i = _quant_weight_int(np.asarray(w1, np.float32))
    w2i = _quant_weight_int(np.asarray(w2, np.float32))
    w2t = _lhsT_taps(w2i)  # [C, 9*C], tap-major (t9 = (dy+1)*3 + dx+1)
    tap = lambda t9: w2t[:, t9 * C:(t9 + 1) * C]
    blocks = []
    for dxi in range(3):           # DR pairs: (dy=-1,dx) then (dy=+1,dx)
        blocks += [tap(dxi), tap(6 + dxi)]
    blocks += [tap(3), tap(5)]     # DR pair: (dy=0,dx=-1) + (dy=0,dx=+1)
    blocks.append(tap(4))          # single: (dy=0,dx=0)
    w2p = np.concatenate(blocks, axis=1).astype(ml_dtypes.float8_e4m3)

    s1, b1 = _bn_affine(np.asarray(gamma1, np.float32), np.asarray(beta1, np.float32),
                        np.asarray(mean1, np.float32), np.asarray(var1, np.float32))
    s2, b2 = _bn_affine(np.asarray(gamma2, np.float32), np.asarray(beta2, np.float32),
                        np.asarray(mean2, np.float32), np.asarray(var2, np.float32))
    # conv PSUM holds 225*conv (15x-or-15a input, 15w weights) -> want 15*bn:
    bnv = np.stack([s1 / 15.0, 15.0 * b1, s2 / 15.0, 15.0 * b2],
                   axis=1).astype(np.float32)  # [C, 4]

    w1r = _lhsT_taps(w1i).astype(np.float32)
    shared = {"w2p": w2p, "w1r": w1r, "bnv": bnv}
    return [{"x15": np.ascontiguousarray(x15[i]), **shared}
            for i in range(N_CORES)]


def kernel(**inputs):
    from concourse.bass_utils import run_bass_kernel_spmd

    nc = _get_module()
    in_maps = _make_in_maps(**inputs)
    res = run_bass_kernel_spmd(nc, in_maps, core_ids=list(range(N_CORES)))
    _CACHE["last_res"] = res
    out = np.concatenate([np.asarray(r["out"], np.float32)
                          for r in res.results], axis=0)
    return out.reshape(B, C, H, W)


# revision 38
# speedup vs baseline: 1.0072x; 1.0035x over previous
"""Trainium2 Bass kernel for a DoReFa-quantized ResNet BasicBlock (inference).

Reference computation (all fp32):
    out = qact(bn2(conv3x3(qact(bn1(conv3x3(x, qw(w1)))), qw(w2))) + x)
with qw = 4-bit DoReFa weight quant, qact = 4-bit activation quant,
x: (64, 128, 56, 56), convs 128->128 stride 1 pad 1.

Sharding: data-parallel over the batch dim, 8 images per NeuronCore on 8 cores.

Per-core kernel design:
  * NCHW with C=128 on SBUF partitions, flattened zero-padded image rows in
    the free dim; a 3x3 conv = shifted 128x128 matmuls accumulated in PSUM
    (8-row chunks, one PSUM bank each).
  * Dual row pitches: conv1/x use 58-wide rows (464-col matmuls, minimal
    padding); act1/conv2 use 64-wide rows so the fp8 DoubleRow pair stride
    (2*64 = 128 B) satisfies the %16 rule. The activation-quantize round op
    bridges the two layouts for free via strided APs.
  * Quantized weights are exact small integers (15*w_q odd in [-15,15]) and
    activations are 15*a in {0..15} (exact in fp8e4m3) -> conv2 is bit-exact
    integer arithmetic in 5 matmuls per chunk: 3 fp8 DoubleRow matmuls for
    the (dy=-1,+1) tap pairs, a 4th DoubleRow pairing (dy=0,dx=-1)+(dx=+1)
    against a 16-byte-aligned shifted duplicate of act1 (produced by a second
    VectorE round-op), and 1 normal fp8 matmul for the center tap.
  * Conv1 runs in the PE's float32r mode (fp32 exponent, 12-bit significand,
    round-to-nearest; probed on HW) at 1 col/cycle - 4x faster than fp32.
  * BN folds to a per-channel affine applied by ScalarE out of PSUM; DoReFa
    staircase = tensor_scalar clip (max,min) + round-half-even via the +2^23
    fp32 trick on VectorE (bit-matches jnp.round).
  * Software-pipelined emission (conv1 of image n+1 ahead of conv2 of image
    n); all HBM transfers contiguous (staging tiles + VectorE pad insert);
    ~30 warm-up matmuls during the head DMA window pre-trip the PE HAM clock
    gate. A post-Tile pass splits multi-semaphore waits onto same-engine
    NoOps (this walrus encodes at most one sync wait per instruction).

Measured (8 cores, NTFF profile): ~204 us HW exec, rel L2 err ~8e-3
(~0.7% of outputs off by one 1/15 quantization step; PE >99% packed in its
window; 98 matmuls per image per core).
"""

import os
import sys

import numpy as np

for _p in ("/opt/trn_rl_repo", "/opt/pypackages"):
    if _p not in sys.path and os.path.isdir(_p):
        sys.path.insert(0, _p)

import ml_dtypes  # noqa: E402

# ---------------------------------------------------------------- constants
B, C, H, W = 64, 128, 56, 56
N_CORES = 8
BPC = B // N_CORES          # images per core
WP = W + 2                  # conv1/x padded row length (58)
WP2 = 64                    # conv2/act1 padded row length (58 used + 6 dead; 2*WP2 % 16 == 0 for DoubleRow)
HPAD = H + 2                # padded rows        (58)
IMG = WP * HPAD             # x-layout padded image elems (3364)
IMG2 = WP2 * HPAD           # act1-layout padded image elems (3712)
BUF = IMG + 4               # x/v1 buffer
BUF2 = IMG2 + 4             # act1/v2 buffer
ACT_D = 3726                # shifted act1 copy offset; pair step D+2 %16==0
ABUF = ACT_D + BUF2         # act1 tile width (original + shifted copy)
XB = 1                      # x / v / out buffers: image base offset
AB = 2                      # act1 buffer: base offset (keeps bf16 dest 4B aligned)
RPC = 8                     # padded rows per PSUM chunk
NCHUNK = H // RPC           # 7 chunks cover output rows 1..56
FREE = RPC * WP             # 464 free elems per conv1 matmul
FREE2 = RPC * WP2           # 512 free elems per conv2 matmul (one PSUM bank)
MAGIC = float(2**23)        # fp32 round-to-nearest-even magic constant
EPS = 1e-5

# conv1 input mode: "f32r" = single fp32r matmul per tap (fast; reduced-
# precision PE mode), "hilo" = fp16 hi+lo split (2 matmuls per tap, ~2^-22).
CONV1_MODE = os.environ.get("K_CONV1_MODE", "f32r")

_CACHE = {}


# ---------------------------------------------------------------- host math
def _quant_weight_int(w):
    """Return 15*quantize_weight(w, 4) which is an exact odd integer in
    [-15, 15], as float32. Mirrors reference elementwise fp32 ops; tanh is
    computed in f64 and rounded (closest to any correctly-rounded f32 tanh)."""
    wt = np.tanh(w.astype(np.float64)).astype(np.float32)
    m = np.float32(np.abs(wt).max())
    wtn = wt / (np.float32(2.0) * m) + np.float32(0.5)      # [0, 1]
    q = np.round(wtn * np.float32(15.0)).astype(np.float32)  # {0..15}, half-even
    return np.float32(2.0) * q - np.float32(15.0)            # odd ints [-15,15]


def _bn_affine(gamma, beta, mean, var):
    """Per-channel (scale, bias) with bn(y) = scale*y + bias, in f64."""
    inv = 1.0 / np.sqrt(var.astype(np.float64) + EPS)
    s = gamma.astype(np.float64) * inv
    b = beta.astype(np.float64) - mean.astype(np.float64) * s
    return s, b


def _lhsT_taps(w_int):
    """[oc, ic, 3, 3] -> [ic, 9*oc] stationary-operand layout (tap-major)."""
    # lhsT for tap t lives at columns [t*128, (t+1)*128), laid out [ic, oc]
    t = np.transpose(w_int, (2, 3, 1, 0)).reshape(9, C, C)   # [tap, ic, oc]
    return np.transpose(t, (1, 0, 2)).reshape(C, 9 * C)


# ---------------------------------------------------------------- bass build
def _split_multiwaits(nc, mybir):
    """Walrus in this toolchain encodes at most ONE sync wait per instruction.

    Tile emits instructions with several on_wait entries; hoist all but one
    onto same-engine NoOps placed immediately before the instruction (the
    sequencer executes them in order, so semantics are unchanged)."""
    nid = 0
    for fn in nc.m.functions:
        for blk in fn.blocks:
            out = []
            changed = False
            for ins in blk.instructions:
                si = ins.sync_info
                if si is not None and len(si.on_wait) > 1:
                    waits = list(si.on_wait)
                    for w in waits[:-1]:
                        nid += 1
                        nop = mybir.InstNoOp(name=f"I-wfix-{nid}",
                                             engine=ins.engine)
                        nop.sync_info = mybir.SyncInfo(on_wait=[w],
                                                       on_update=[])
                        out.append(nop)
                    ins.sync_info = mybir.SyncInfo(
                        on_wait=[waits[-1]], on_update=list(si.on_update))
                    changed = True
                out.append(ins)
            if changed:
                blk.instructions = out


def _build_module(apply_wfix=True):
    import concourse.bass as bass
    import concourse.mybir as mybir
    import concourse.tile as tile
    from contextlib import ExitStack

    f32 = mybir.dt.float32
    f16 = mybir.dt.float16
    bf16 = mybir.dt.bfloat16
    AF = mybir.ActivationFunctionType
    OP = mybir.AluOpType

    nc = bass.Bass("TRN2", target_bir_lowering=False, debug=False,
                   num_devices=N_CORES)

    f32r = mybir.dt.float32r
    u32 = mybir.dt.uint32

    x_dtype = f32r if CONV1_MODE == "f32r" else f32
    x_d = nc.dram_tensor("x15", [BPC, C, H, W], x_dtype, kind="ExternalInput")
    # conv2 weights fp8: 3 DoubleRow pair blocks [2,128] (dy=-1/+1 per dx),
    # then 3 dy=0 single taps
    f8 = mybir.dt.float8e4
    w2p_d = nc.dram_tensor("w2p", [C, 9 * C], f8, kind="ExternalInput")
    # conv1 weights as f32 (for the f32r matmul path)
    w1r_d = nc.dram_tensor("w1r", [C, 9 * C], f32r, kind="ExternalInput")
    # columns: [sc1, bi1, sc2, bi2]
    bn_d = nc.dram_tensor("bnv", [C, 4], f32, kind="ExternalInput")
    out_d = nc.dram_tensor("out", [BPC, C, H, W], f32, kind="ExternalOutput")

    lo = XB + WP            # first valid (row 1) element in XB-based buffers
    hi = XB + (HPAD - 1) * WP  # one past row 56 (= start of pad row 57)

    with tile.TileContext(nc) as tc, ExitStack() as ctx:
        const = ctx.enter_context(tc.tile_pool(name="const", bufs=1))
        sb = ctx.enter_context(tc.tile_pool(name="sb", bufs=2))
        xp = ctx.enter_context(tc.tile_pool(name="xp", bufs=3))
        ps = ctx.enter_context(tc.tile_pool(name="ps", bufs=4, space="PSUM"))

        # order: conv1 weights first (first-matmul critical path), then BN
        # vectors, then conv2 weights (not needed until ~40us in)
        w1r_sb = const.tile([C, 9 * C], f32r)
        if CONV1_MODE == "f32r":
            for q in range(3):
                nc.sync.dma_start(w1r_sb[:, q * 3 * C:(q + 1) * 3 * C],
                                  w1r_d.ap()[:, q * 3 * C:(q + 1) * 3 * C])
        bn_sb = const.tile([C, 4], f32)
        nc.sync.dma_start(bn_sb[:], bn_d.ap())
        w2p_sb = const.tile([C, 9 * C], f8)
        nc.sync.dma_start(w2p_sb[:], w2p_d.ap())
        if CONV1_MODE == "f32r":
            # HAM warm-up: ~30 throwaway matmuls on the just-landed weights
            # fill the otherwise-idle head so real matmuls start at 2.4 GHz
            pwarm = ps.tile([C, FREE], f32, tag="p1", name="pwarm")
            for _ in range(30):
                nc.tensor.matmul(pwarm[:, 0:C], lhsT=w1r_sb[:, 0:C],
                                 rhs=w1r_sb[:, 0:C], start=True, stop=True)
        sc1_sb = bn_sb[:, 0:1]
        bi1_sb = bn_sb[:, 1:2]
        sc2_sb = bn_sb[:, 2:3]
        bi2_sb = bn_sb[:, 3:4]

        def emit_load_conv1(n):
            """Load image n, run conv1 + bn1 + qact; returns (x, act1)."""
            x = xp.tile([C, BUF], f32, tag="x", name=f"x_{n}")
            xw = x.bitcast(f32r) if CONV1_MODE == "f32r" else x
            xwr = xw[:, XB:XB + IMG].rearrange("p (h w) -> p h w", w=WP)
            # zero pad borders + slack (everything outside the DMA interior)
            xr0 = x[:, XB:XB + IMG].rearrange("p (h w) -> p h w", w=WP)
            nc.gpsimd.memset(x[:, 0:XB + WP + 1], 0.0)          # slack+row0
            nc.gpsimd.memset(x[:, XB + (HPAD - 1) * WP:BUF], 0.0)  # row57+slack
            nc.gpsimd.memset(xr0[:, 1:57, 0], 0.0)               # left pad col
            nc.gpsimd.memset(xr0[:, 1:57, 57], 0.0)              # right pad col
            # contiguous quarter DMAs into staging (128 descriptors each,
            # vs 1792 for strided pad-layout writes), then DVE pad-insert
            xs = sb.tile([C, H * W], x_dtype, tag="xs", name=f"xs_{n}")
            xsr = xs.rearrange("p (h w) -> p h w", w=W)
            xd_flat = x_d.ap()[n].rearrange("p h w -> p (h w)")
            for q in range(4):
                r0, r1 = 1 + 14 * q, 15 + 14 * q
                nc.sync.dma_start(xs[:, (r0 - 1) * W:(r1 - 1) * W],
                                  xd_flat[:, (r0 - 1) * W:(r1 - 1) * W])
                nc.vector.tensor_copy(
                    xwr[:, r0:r1, 1:57],
                    xsr[:, r0 - 1:r1 - 1, :])

            v1 = sb.tile([C, BUF], f32, tag="v1", name=f"v1_{n}")
            for cch in range(NCHUNK):
                p1 = ps.tile([C, FREE], f32, tag="p1", name=f"p1_{n}_{cch}")
                r0 = 1 + RPC * cch
                for t9 in range(9):
                    dy, dx = t9 // 3 - 1, t9 % 3 - 1
                    off = XB + (r0 + dy) * WP + dx
                    nc.tensor.matmul(
                        p1[:],
                        lhsT=w1r_sb[:, t9 * C:(t9 + 1) * C],
                        rhs=xw[:, off:off + FREE],
                        start=(t9 == 0), stop=(t9 == 8))
                dst = v1[:, XB + r0 * WP:XB + r0 * WP + FREE]
                nc.scalar.activation(dst, p1[:], AF.Identity,
                                     bias=bi1_sb, scale=sc1_sb)

            # qact: clip to [0,15] then round (kept as 15*act, fp8 exact).
            # The round op also re-layouts W58 (v1) -> W64 (act1) rows.
            nc.vector.tensor_scalar(v1[:, lo:hi], v1[:, lo:hi],
                                    0.0, 15.0, op0=OP.max, op1=OP.min)
            act1 = sb.tile([C, ABUF], f8, tag="act1", name=f"act1_{n}")
            v1r = v1[:, XB:XB + IMG].rearrange("p (h w) -> p h w", w=WP)
            ar = act1[:, AB:AB + IMG2].rearrange("p (h w) -> p h w", w=WP2)
            nc.vector.tensor_scalar(ar[:, 1:57, 0:WP], v1r[:, 1:57, 0:WP],
                                    MAGIC, MAGIC, op0=OP.add, op1=OP.subtract)
            nc.gpsimd.memset(act1[:, 0:AB + WP2 + 1], 0.0)
            nc.gpsimd.memset(act1[:, AB + (HPAD - 1) * WP2:BUF2], 0.0)
            nc.gpsimd.memset(ar[:, 1:57, 0], 0.0)
            nc.gpsimd.memset(ar[:, 1:57, 57:64], 0.0)
            # shifted duplicate of act1 (for the dy=0 DoubleRow pair):
            # second round-op writing at +ACT_D, plus its own zero borders
            ar2 = act1[:, ACT_D + AB:ACT_D + AB + IMG2].rearrange(
                "p (h w) -> p h w", w=WP2)
            nc.vector.tensor_scalar(ar2[:, 1:57, 0:WP], v1r[:, 1:57, 0:WP],
                                    MAGIC, MAGIC, op0=OP.add, op1=OP.subtract)
            nc.gpsimd.memset(act1[:, ACT_D:ACT_D + AB + WP2 + 1], 0.0)
            nc.gpsimd.memset(act1[:, ACT_D + AB + (HPAD - 1) * WP2:ABUF], 0.0)
            nc.gpsimd.memset(ar2[:, 1:57, 0], 0.0)
            nc.gpsimd.memset(ar2[:, 1:57, 57:64], 0.0)
            return x, act1

        def emit_conv2_out(n, x, act1):
            """conv2 + bn2 + residual + qact for image n, DMA result out."""
            v2 = sb.tile([C, BUF2], f32, tag="v2", name=f"v2_{n}")
            inv15 = float(np.float32(1.0) / np.float32(15.0))
            vr = v2[:, XB:XB + IMG2].rearrange("p (h w) -> p h w", w=WP2)
            xr = x[:, XB:XB + IMG].rearrange("p (h w) -> p h w", w=WP)
            ost = sb.tile([C, H * W], f32, tag="ost", name=f"ost_{n}")
            ostr = ost.rearrange("p (h w) -> p h w", w=W)
            od_flat = out_d.ap()[n].rearrange("p h w -> p (h w)")

            def emit_quarter(q):
                # residual + qact + /15 + out-DMA for output rows 14q..14q+14;
                # emitted right after the last PSUM chunk covering them so the
                # scheduler drains the elementwise tail during the MM stream
                r0, r1 = 1 + 14 * q, 15 + 14 * q
                vq = vr[:, r0:r1, 0:WP]      # [14, 58] rows of the W64 buffer
                nc.vector.tensor_add(vq, vq, xr[:, r0:r1, 0:WP])
                # clip+round+scale split across engines: ScalarE takes the
                # max(.,0) half (Relu), DVE fuses min/+M and -M/(*1/15)
                nc.scalar.activation(vq, vq, AF.Relu)
                nc.vector.tensor_scalar(vq, vq, 15.0, MAGIC,
                                        op0=OP.min, op1=OP.add)
                nc.vector.tensor_scalar(ostr[:, r0 - 1:r1 - 1, :],
                                        vr[:, r0:r1, 1:57], MAGIC, inv15,
                                        op0=OP.subtract, op1=OP.mult)
                nc.sync.dma_start(od_flat[:, (r0 - 1) * W:(r1 - 1) * W],
                                  ost[:, (r0 - 1) * W:(r1 - 1) * W])

            # quarter q is ready once chunk ceil((14*(q+1))/8)-1 is done
            quarter_after = {1: 0, 3: 1, 5: 2, 6: 3}
            for cch in range(NCHUNK):
                p2 = ps.tile([C, FREE2], f32, tag="p2", name=f"p2_{n}_{cch}")
                r0 = 1 + RPC * cch
                for dxi, dx in enumerate((-1, 0, 1)):
                    # DoubleRow: taps (dy=-1,dx) + (dy=+1,dx) in one matmul;
                    # pair stride = 2*WP2 = 128 fp8 bytes (%16 == 0)
                    off_a = AB + (r0 - 1) * WP2 + dx
                    mv = bass.AP(tensor=act1.tensor, offset=off_a,
                                 ap=[[ABUF, C], [2 * WP2, 2], [1, FREE2]])
                    wpair = w2p_sb[:, dxi * 2 * C:(dxi + 1) * 2 * C].rearrange(
                        "p (two m) -> p two m", two=2)
                    nc.tensor.matmul(p2[:], lhsT=wpair, rhs=mv,
                                     perf_mode=mybir.MatmulPerfMode.DoubleRow,
                                     start=(dxi == 0), stop=False)
                # 4th DoubleRow: (dy=0,dx=-1) from the original + (dy=0,dx=+1)
                # from the shifted copy -> pair step ACT_D + 2 (16-aligned)
                off_a = AB + r0 * WP2 - 1
                mv = bass.AP(tensor=act1.tensor, offset=off_a,
                             ap=[[ABUF, C], [ACT_D + 2, 2], [1, FREE2]])
                wpair = w2p_sb[:, 6 * C:8 * C].rearrange(
                    "p (two m) -> p two m", two=2)
                nc.tensor.matmul(p2[:], lhsT=wpair, rhs=mv,
                                 perf_mode=mybir.MatmulPerfMode.DoubleRow,
                                 start=False, stop=False)
                # remaining single: (dy=0, dx=0)
                off = AB + r0 * WP2
                nc.tensor.matmul(p2[:], lhsT=w2p_sb[:, 8 * C:9 * C],
                                 rhs=act1[:, off:off + FREE2],
                                 start=False, stop=True)
                dst = v2[:, XB + r0 * WP2:XB + r0 * WP2 + FREE2]
                nc.scalar.activation(dst, p2[:], AF.Identity,
                                     bias=bi2_sb, scale=sc2_sb)
                if cch in quarter_after:
                    emit_quarter(quarter_after[cch])

            # + residual (x buffer holds 15*x), then qact, then /15 —
            # in two half-image pieces so the first out-DMA overlaps the
            # second half's elementwise tail.


        prev = None
        for s in range(BPC + 1):
            cur = emit_load_conv1(s) if s < BPC else None
            if prev is not None:
                emit_conv2_out(s - 1, *prev)
            prev = cur

    if apply_wfix:
        _split_multiwaits(nc, mybir)
    return nc


def _get_module(apply_wfix=True):
    key = ("nc", apply_wfix)
    if key not in _CACHE:
        _CACHE[key] = _build_module(apply_wfix)
    return _CACHE[key]


# ---------------------------------------------------------------- host entry
def _make_in_maps(x, w1, w2, gamma1, beta1, mean1, var1,
                  gamma2, beta2, mean2, var2):
    x15 = (np.float32(15.0) * np.asarray(x, np.float32))
    x15 = x15.reshape(N_CORES, BPC, C, H, W)

    w1i = _quant_weight_int(np.asarray(w1, np.float32))
    w2i = _quant_weight_int(np.asarray(w2, np.float32))
    w2t = _lhsT_taps(w2i)  # [C, 9*C], tap-major (t9 = (dy+1)*3 + dx+1)
    tap = lambda t9: w2t[:, t9 * C:(t9 + 1) * C]
    blocks = []
    for dxi in range(3):           # DR pairs: (dy=-1,dx) then (dy=+1,dx)
        blocks += [tap(dxi), tap(6 + dxi)]
    blocks += [tap(3), tap(5)]     # DR pair: (dy=0,dx=-1) + (dy=0,dx=+1)
    blocks.append(tap(4))          # single: (dy=0,dx=0)
    w2p = np.concatenate(blocks, axis=1).astype(ml_dtypes.float8_e4m3)

    s1, b1 = _bn_affine(np.asarray(gamma1, np.float32), np.asarray(beta1, np.float32),
                        np.asarray(mean1, np.float32), np.asarray(var1, np.float32))
    s2, b2 = _bn_affine(np.asarray(gamma2, np.float32), np.asarray(beta2, np.float32),
                        np.asarray(mean2, np.float32), np.asarray(var2, np.float32))
    # conv PSUM holds 225*conv (15x-or-15a input, 15w weights) -> want 15*bn:
    bnv = np.stack([s1 / 15.0, 15.0 * b1, s2 / 15.0, 15.0 * b2],
                   axis=1).astype(np.float32)  # [C, 4]

    w1r = _lhsT_taps(w1i).astype(np.float32)
    shared = {"w2p": w2p, "w1r": w1r, "bnv": bnv}
    return [{"x15": np.ascontiguousarray(x15[i]), **shared}
            for i in range(N_CORES)]


def kernel(**inputs):
    from concourse.bass_utils import run_bass_kernel_spmd

    nc = _get_module()
    in_maps = _make_in_maps(**inputs)
    res = run_bass_kernel_spmd(nc, in_maps, core_ids=list(range(N_CORES)))
    _CACHE["last_res"] = res
    out = np.concatenate([np.asarray(r["out"], np.float32)
                          for r in res.results], axis=0)
    return out.reshape(B, C, H, W)
